# revision 1
# baseline (speedup 1.0000x reference)
"""Trainium2 Bass kernel for CapsuleLayer (nn_CapsuleLayer_45552423142009).

Computes, for x[B,768]:
  u = squash(x @ Wp + bp)            # [B, 8, 16]  (squash over last dim)
  u_hat[b,p,c,:] = u[b,p,:] @ W[p,c] # [B, 8, 5, 16]
  3 iterations of dynamic routing -> v [B, 5, 16]

Strategy: pure data-parallel over 8 NeuronCores (batch sharded 16384/core).
On-chip layout is "transposed": features on partitions, batch on the free
dim (512-wide tiles).  PE does transposes of x, the two big matmuls and all
broadcast / segment-sum reductions (via 0/1 selector matrices, fp32r at
1 cycle/row).  DVE/ACT/GPSIMD do the elementwise work.
"""

import sys
import numpy as np

sys.path.insert(0, "/opt/trn_rl_repo")

from concourse import bass, bacc, mybir  # noqa: E402
from concourse import tile  # noqa: E402
from concourse.bass_utils import run_bass_kernel_spmd  # noqa: E402
from concourse.alu_op_type import AluOpType  # noqa: E402

F32 = mybir.dt.float32
F32R = mybir.dt.float32r
AF = mybir.ActivationFunctionType

B = 131072
D = 768
P = 8
PD = 16
C = 5
CD = 16
NCORES = 8
BC = B // NCORES          # 16384 batch rows per core
NB = 512                  # batch columns per tile
NT = BC // NB             # 32 tiles

# selector blob column offsets
SEL_SSEL8 = 0      # [128, 8]   sum o-groups of 16 -> p
SEL_PSEL16 = 8     # [128, 16]  0.2 * (sum over p at fixed j)
SEL_PSEL8 = 24     # [128, 8]   sum over j at fixed p
SEL_IDENT = 32     # [128, 128] identity
SEL_TILE8 = 160    # [16, 128]  broadcast j -> (p, j)
SEL_SBC = 288      # [8, 128]   broadcast p -> (p, o)
SEL_JSEL = 416     # [80, 5]    sum over j at fixed c
SEL_JBC = 421      # [5, 80]    broadcast c -> (c, j)
SEL_CSEL = 501     # [40, 8]    sum over c at fixed p   (logits layout (c,p))
SEL_CBC = 509      # [8, 40]    broadcast p -> (c, p)
SEL_BSEL = 549     # [40, 640]  5 x [40,128]: broadcast (c,p) -> (p,i) for class c
SEL_ESEL = 1190    # [16, 40]   5 x [16,8]: col c ones (vsq accumulate)
SEL_GBC = 1230     # [8, 80]    5 x [8,16]: row c ones (g -> j-bcast, class c)
SEL_GBC40 = 1310   # [8, 40]    [c', (c,p)] = d_c'c (g -> (c,p) bcast)
SEL_ASEL = 1350    # [128, 200] 5 x [128,40]: [(p,j),(c',p')] = d_pp' d_c'c
SEL_W = 1550
CST_W = SEL_W + 768 + 640 + 80 + 1


def _r(ap):
    return ap.bitcast(F32R)


def build_selectors() -> np.ndarray:
    sel = np.zeros((128, SEL_W), dtype=np.float32)
    for m in range(128):
        sel[m, SEL_SSEL8 + m // 16] = 1.0                      # Ssel8
    for p in range(P):
        for j in range(PD):
            sel[p * 16 + j, SEL_PSEL16 + j] = 0.2              # Psel16 (x0.2)
            sel[p * 16 + j, SEL_PSEL8 + p] = 1.0               # Psel8
    sel[:, SEL_IDENT:SEL_IDENT + 128] = np.eye(128, dtype=np.float32)
    for j in range(16):
        for p in range(P):
            sel[j, SEL_TILE8 + p * 16 + j] = 1.0               # Tile8
    for p in range(P):
        sel[p, SEL_SBC + p * 16:SEL_SBC + (p + 1) * 16] = 1.0  # Sbc
    for c in range(C):
        for j in range(CD):
            sel[c * 16 + j, SEL_JSEL + c] = 1.0                # Jsel
            sel[c, SEL_JBC + c * 16 + j] = 1.0                 # Jbc
    for c in range(C):
        for p in range(P):
            sel[c * 8 + p, SEL_CSEL + p] = 1.0                 # Csel
            sel[p, SEL_CBC + c * 8 + p] = 1.0                  # Cbc
            # Bsel_c: [(c',p), (p',i)] = d_cc' d_pp'
            sel[c * 8 + p, SEL_BSEL + c * 128 + p * 16:
                SEL_BSEL + c * 128 + (p + 1) * 16] = 1.0
    for c in range(C):
        for j in range(CD):
            sel[j, SEL_ESEL + c * 8 + c] = 1.0                 # Esel_c col c
            sel[c, SEL_GBC + c * 16 + j] = 1.0                 # Gbc_c row c
    for c in range(C):
        for p in range(P):
            sel[c, SEL_GBC40 + c * 8 + p] = 1.0                # Gbc40
    for c in range(C):
        for p in range(P):
            for j in range(CD):
                # Asel_c: [(p,j), (c',p')] = d_pp' d_c'c
                sel[p * 16 + j, SEL_ASEL + c * 40 + c * 8 + p] = 1.0
    return sel


def build_nc(nt: int = NT) -> bass.Bass:
    bc = nt * NB
    nc = bacc.Bacc(None)

    x_d = nc.declare_dram_parameter("xc", [bc, D], F32R, isOutput=False)
    cst_d = nc.declare_dram_parameter("cst", [128, CST_W], F32R, isOutput=False)
    v_d = nc.declare_dram_parameter("vout", [bc, C * CD], F32, isOutput=True)

    with tile.TileContext(nc) as tc, nc.allow_low_precision(reason="float32r matmul inputs"):
        with (
            tc.sbuf_pool(name="const", bufs=1) as cpool,
            tc.sbuf_pool(name="xin", bufs=2) as xpool,
            tc.sbuf_pool(name="xt", bufs=2) as xtpool,
            tc.sbuf_pool(name="mid", bufs=2) as mpool,
            tc.sbuf_pool(name="uh", bufs=2) as uhpool,
            tc.sbuf_pool(name="rt", bufs=2) as rtpool,
            tc.sbuf_pool(name="sm", bufs=3) as smpool,
            tc.psum_pool(name="pxt", bufs=1) as pxt,
            tc.psum_pool(name="puh", bufs=2) as puhp,
            tc.psum_pool(name="pbc", bufs=2) as pbcp,
            tc.psum_pool(name="pmid", bufs=1) as pmidp,
            tc.psum_pool(name="psm", bufs=2) as psmp,
        ):
            # ---- load constants (one DMA), then stage through DVE so every
            # consumer depends on the DVE semaphore (merges with data deps;
            # walrus allows only ~2 distinct sync waits per instruction) ----
            cst0 = cpool.tile([128, CST_W], F32R)
            nc.sync.dma_start(out=cst0[:], in_=cst_d[:])
            cst = cpool.tile([128, CST_W], F32R)
            nc.vector.tensor_copy(cst[:], cst0[:])
            sel_sb = cst[:, 0:SEL_W]
            wp_sb = cst[:, SEL_W:SEL_W + 768]
            wbd_sb = cst[:, SEL_W + 768:SEL_W + 1408]
            wflat_sb = cst[:, SEL_W + 1408:SEL_W + 1488]
            bp_sb = cst[:, SEL_W + 1488:SEL_W + 1489].bitcast(F32)

            ident = sel_sb[:, SEL_IDENT:SEL_IDENT + 128]

            for it in range(nt):
                # ---- load x tile [512, 768] as 4 x [128, 768] ----
                x_sb = xpool.tile([128, 4, 768], F32R, tag="xin")
                src = x_d[it * NB:(it + 1) * NB, :].rearrange(
                    "(q p) d -> p q d", p=128)
                nc.sync.dma_start(out=x_sb[:], in_=src)

                # ---- transpose x -> xT chunks [128(d), 512(b)] x 6 ----
                xT = xtpool.tile([128, 6, NB], F32R, tag="xt")
                for k in range(6):
                    pt = pxt.tile([128, NB], F32R, tag="pxt")
                    for q in range(4):
                        nc.tensor.transpose(
                            _r(pt[:, q * 128:(q + 1) * 128]),
                            _r(x_sb[:, q, k * 128:(k + 1) * 128]),
                            _r(ident),
                        )
                    if k % 2 == 0:
                        nc.vector.tensor_copy(xT[:, k, :], pt[:])
                    else:
                        nc.scalar.copy(xT[:, k, :], pt[:])

                # ---- mm1: u_pre[(p,o), b] = Wp^T x^T  (+bias via ACT) ----
                pu = pmidp.tile([128, NB], F32, tag="pmid")
                for k in range(6):
                    nc.tensor.matmul(
                        pu[:], _r(wp_sb[:, k * 128:(k + 1) * 128]),
                        _r(xT[:, k, :]), start=(k == 0), stop=(k == 5))
                u_pre = mpool.tile([128, NB], F32, tag="mid")
                nc.scalar.activation(u_pre[:], pu[:], AF.Identity,
                                     bias=bp_sb[:], scale=1.0)

                # ---- squash factor f[p, b] ----
                usq = mpool.tile([128, NB], F32R, tag="mid2")
                nc.gpsimd.tensor_mul(usq[:], u_pre[:], u_pre[:])
                psq = psmp.tile([8, NB], F32, tag="psm")
                nc.tensor.matmul(psq[:], _r(sel_sb[:, SEL_SSEL8:SEL_SSEL8 + 8]),
                                 _r(usq[:]), start=True, stop=True)
                srt = smpool.tile([8, NB], F32, tag="sm")
                nc.scalar.sqrt(srt[:], psq[:])
                sq1 = smpool.tile([8, NB], F32, tag="sm")
                nc.scalar.add(sq1[:], psq[:], 1.0)
                den = smpool.tile([8, NB], F32, tag="sm")
                # den = (srt + 1e-8) * sq1
                nc.vector.scalar_tensor_tensor(
                    den[:], srt[:], 1e-8, sq1[:],
                    op0=AluOpType.add, op1=AluOpType.mult)
                rden = smpool.tile([8, NB], F32, tag="sm")
                nc.vector.reciprocal(rden[:], den[:])
                fz = smpool.tile([8, NB], F32R, tag="sm")
                nc.vector.tensor_mul(fz[:], psq[:], rden[:])
                pfb = pbcp.tile([128, NB], F32, tag="pbc")
                nc.tensor.matmul(pfb[:], _r(sel_sb[:8, SEL_SBC:SEL_SBC + 128]),
                                 _r(fz[:]), start=True, stop=True)
                u = mpool.tile([128, NB], F32R, tag="mid3")
                nc.vector.tensor_mul(u[:], u_pre[:], pfb[:])

                # ---- u_hat_c = Wbd_c^T u   (5 psum banks -> sbuf) ----
                uh = []
                for c in range(C):
                    puh = puhp.tile([128, NB], F32, tag="puh")
                    nc.tensor.matmul(
                        puh[:], _r(wbd_sb[:, c * 128:(c + 1) * 128]),
                        _r(u[:]), start=True, stop=True)
                    uhc = uhpool.tile([128, NB], F32R, tag=f"uh{c}")
                    if c % 2 == 0:
                        nc.scalar.copy(uhc[:], puh[:])
                    else:
                        nc.vector.tensor_copy(uhc[:], puh[:])
                    uh.append(uhc)

                # ---- routing ----
                logit = None
                v5 = None
                for itr in range(3):
                    if itr > 0:
                        e = rtpool.tile([40, NB], F32R, tag="rt_e")
                        nc.scalar.activation(e[:], logit[:], AF.Exp)
                        pden = psmp.tile([8, NB], F32, tag="psm")
                        nc.tensor.matmul(
                            pden[:], _r(sel_sb[:40, SEL_CSEL:SEL_CSEL + 8]),
                            _r(e[:]), start=True, stop=True)
                        rd = smpool.tile([8, NB], F32R, tag="sm")
                        nc.vector.reciprocal(rd[:], pden[:])
                        pdb = pbcp.tile([40, NB], F32, tag="pbc")
                        nc.tensor.matmul(
                            pdb[:], _r(sel_sb[:8, SEL_CBC:SEL_CBC + 40]),
                            _r(rd[:]), start=True, stop=True)
                        cn = rtpool.tile([40, NB], F32R, tag="rt_cn")
                        nc.vector.tensor_mul(cn[:], e[:], pdb[:])

                    # s[j, c, b] per class via matmul; copy to sbuf (rounded)
                    s_sb = rtpool.tile([16, 5, NB], F32R, tag="rt_s")
                    for c in range(C):
                        psc = psmp.tile([16, NB], F32, tag="psm")
                        if itr == 0:
                            nc.tensor.matmul(
                                psc[:],
                                _r(sel_sb[:, SEL_PSEL16:SEL_PSEL16 + 16]),
                                _r(uh[c][:]), start=True, stop=True)
                        else:
                            pcb = pbcp.tile([128, NB], F32, tag="pbc")
                            nc.tensor.matmul(
                                pcb[:],
                                _r(sel_sb[:40, SEL_BSEL + c * 128:
                                          SEL_BSEL + (c + 1) * 128]),
                                _r(cn[:]), start=True, stop=True)
                            t = rtpool.tile([128, NB], F32R, tag="rt_t")
                            nc.vector.tensor_mul(t[:], u[:], pcb[:])
                            nc.tensor.matmul(
                                psc[:],
                                _r(wflat_sb[:, c * 16:(c + 1) * 16]),
                                _r(t[:]), start=True, stop=True)
                        if c % 2 == 0:
                            nc.scalar.copy(s_sb[:, c, :], psc[:])
                        else:
                            nc.vector.tensor_copy(s_sb[:, c, :], psc[:])

                    # vsq[c, b] = sum_j s^2 via accumulating one-hot matmuls
                    ssq = rtpool.tile([16, 5, NB], F32R, tag="rt_ssq")
                    nc.gpsimd.tensor_mul(ssq[:], s_sb[:], s_sb[:])
                    pvq = psmp.tile([8, NB], F32, tag="psm")
                    for c in range(C):
                        nc.tensor.matmul(
                            pvq[:], _r(sel_sb[:16, SEL_ESEL + c * 8:
                                              SEL_ESEL + (c + 1) * 8]),
                            _r(ssq[:, c, :]), start=(c == 0), stop=(c == 4))
                    # g = vsq / ((1+vsq) (sqrt(vsq)+1e-8))
                    vsrt = smpool.tile([8, NB], F32, tag="sm")
                    nc.scalar.sqrt(vsrt[:], pvq[:])
                    vsq1 = smpool.tile([8, NB], F32, tag="sm")
                    nc.scalar.add(vsq1[:], pvq[:], 1.0)
                    vden = smpool.tile([8, NB], F32, tag="sm")
                    nc.vector.scalar_tensor_tensor(
                        vden[:], vsrt[:], 1e-8, vsq1[:],
                        op0=AluOpType.add, op1=AluOpType.mult)
                    rvd = smpool.tile([8, NB], F32, tag="sm")
                    nc.vector.reciprocal(rvd[:], vden[:])
                    g = smpool.tile([8, NB], F32R, tag="sm")
                    nc.vector.tensor_mul(g[:], pvq[:], rvd[:])

                    if itr < 2:
                        # agreement with v = g*s folded after the j-sum:
                        # atil[(c,p), b] = sum_j uh_c[(p,j),b] * s[j,c,b]
                        pat = pmidp.tile([40, NB], F32, tag="pmid")
                        for c in range(C):
                            pvb = pbcp.tile([128, NB], F32, tag="pbc")
                            nc.tensor.matmul(
                                pvb[:],
                                _r(sel_sb[:16, SEL_TILE8:SEL_TILE8 + 128]),
                                _r(s_sb[:, c, :]), start=True, stop=True)
                            pr = rtpool.tile([128, NB], F32R, tag="rt_pr")
                            nc.vector.tensor_mul(pr[:], uh[c][:], pvb[:])
                            nc.tensor.matmul(
                                pat[:],
                                _r(sel_sb[:, SEL_ASEL + c * 40:
                                          SEL_ASEL + (c + 1) * 40]),
                                _r(pr[:]), start=(c == 0), stop=(c == 4))
                        ats = rtpool.tile([40, NB], F32, tag="rt_ats")
                        nc.scalar.copy(ats[:], pat[:])
                        pg40 = psmp.tile([40, NB], F32, tag="psm")
                        nc.tensor.matmul(
                            pg40[:], _r(sel_sb[:8, SEL_GBC40:SEL_GBC40 + 40]),
                            _r(g[:]), start=True, stop=True)
                        if itr == 0:
                            logit = rtpool.tile([40, NB], F32, tag="rt_lg")
                            nc.vector.tensor_mul(logit[:], ats[:], pg40[:])
                        else:
                            a40 = rtpool.tile([40, NB], F32, tag="rt_a40")
                            nc.vector.tensor_mul(a40[:], ats[:], pg40[:])
                            lg2 = rtpool.tile([40, NB], F32, tag="rt_lg2")
                            nc.vector.tensor_add(lg2[:], logit[:], a40[:])
                            logit = lg2
                    else:
                        # final v[j, c, b] = s * g_bcast
                        v5 = rtpool.tile([16, 5, NB], F32R, tag="rt_v")
                        for c in range(C):
                            pgb = psmp.tile([16, NB], F32, tag="psm")
                            nc.tensor.matmul(
                                pgb[:], _r(sel_sb[:8, SEL_GBC + c * 16:
                                                  SEL_GBC + (c + 1) * 16]),
                                _r(g[:]), start=True, stop=True)
                            nc.vector.tensor_mul(
                                v5[:, c, :], s_sb[:, c, :], pgb[:])

                # ---- transpose v back to [b, (c,j)] and store ----
                vo = rtpool.tile([128, 4, 80], F32, tag="rt_vo")
                for q in range(4):
                    pvt = pbcp.tile([128, 80], F32R, tag="pbc")
                    for c in range(C):
                        nc.tensor.transpose(
                            _r(pvt[:, c * 16:(c + 1) * 16]),
                            _r(v5[:, c, q * 128:(q + 1) * 128]),
                            _r(sel_sb[:16, SEL_IDENT:SEL_IDENT + 16]))
                    if q % 2 == 0:
                        nc.scalar.copy(vo[:, q, :], pvt[:])
                    else:
                        nc.vector.tensor_copy(vo[:, q, :], pvt[:])
                dst = v_d[it * NB:(it + 1) * NB, :].rearrange(
                    "(q p) j -> p q j", p=128)
                nc.sync.dma_start(out=dst, in_=vo[:])

    nc.compile()
    return nc


_NC_CACHE: dict = {}


def _get_nc(nt: int) -> bass.Bass:
    if nt not in _NC_CACHE:
        _NC_CACHE[nt] = build_nc(nt)
    return _NC_CACHE[nt]


def _prep_weights(Wp, bp, W):
    Wp = np.asarray(Wp, np.float32)
    bp = np.asarray(bp, np.float32)
    W = np.asarray(W, np.float32)
    wp_flat = Wp.transpose(1, 0, 2).reshape(768, 128)          # [d, (p,o)]
    wp_h = np.ascontiguousarray(
        wp_flat.reshape(6, 128, 128).transpose(1, 0, 2).reshape(128, 768))
    wbd_h = np.zeros((128, 5, 128), np.float32)
    for p in range(P):
        wbd_h[p * 16:(p + 1) * 16, :, p * 16:(p + 1) * 16] = \
            W[p].transpose(1, 0, 2)                            # [i, c, j]
    wbd_h = np.ascontiguousarray(wbd_h.reshape(128, 640))
    wflat_h = np.ascontiguousarray(
        W.transpose(0, 2, 1, 3).reshape(128, 5 * 16))          # [(p,i), (c,j)]
    bp_h = np.ascontiguousarray(bp.reshape(128, 1))
    sel_h = build_selectors()
    return wp_h, wbd_h, wflat_h, bp_h, sel_h


def pack_consts(Wp, bp, W):
    wp_h, wbd_h, wflat_h, bp_h, sel_h = _prep_weights(Wp, bp, W)
    cst = np.concatenate([sel_h, wp_h, wbd_h, wflat_h, bp_h], axis=1)
    assert cst.shape == (128, CST_W), cst.shape
    return np.ascontiguousarray(cst)


def kernel(x, Wp, bp, W):
    x = np.asarray(x, np.float32)
    cst = pack_consts(Wp, bp, W)
    nc = _get_nc(NT)
    in_maps = [{"xc": np.ascontiguousarray(x[i * BC:(i + 1) * BC]), "cst": cst}
               for i in range(NCORES)]
    res = run_bass_kernel_spmd(nc, in_maps, list(range(NCORES)))
    out = np.concatenate([res.results[i]["vout"] for i in range(NCORES)], axis=0)
    return out.reshape(B, C, CD)



# revision 2
# speedup vs baseline: 1.4861x; 1.4861x over previous
"""Trainium2 Bass kernel for CapsuleLayer (nn_CapsuleLayer_45552423142009).

Computes, for x[B,768]:
  u = squash(x @ Wp + bp)            # [B, 8, 16]  (squash over last dim)
  u_hat[b,p,c,:] = u[b,p,:] @ W[p,c] # [B, 8, 5, 16]
  3 iterations of dynamic routing -> v [B, 5, 16]

Strategy: pure data-parallel over 8 NeuronCores (batch sharded 16384/core).
On-chip layout is "transposed": features on partitions, batch on the free
dim (512-wide tiles).  Key differences vs the v1 kernel:
  - x is transposed + cast to fp16 on the host, so the device does no
    PE transposes of x and reads half the HBM bytes.
  - all matmul operands are fp16 (1 cycle/row on the PE vs ~2 for f32r).
  - squash factors via exp/ln only (f = sqrt(q)/(1+q) = exp(.5*ln q -
    ln(1+q))), so the ACT engine stays on one table set (no ~2.7us
    ACT_TABLE_LOAD thrash), and softmax recip uses the fast DVE approx.
  - s/v are packed [80=(c,j), b] via M=80 accumulating selector matmuls:
    one vsq / g-broadcast / v-multiply per iteration instead of five.
  - routing logits accumulate directly in one pinned PSUM bank across
    iterations (PE start=False accumulation), read in place by Exp.
"""

import sys
import numpy as np

sys.path.insert(0, "/opt/trn_rl_repo")

from concourse import bass, bacc, mybir  # noqa: E402
from concourse import tile  # noqa: E402
from concourse.bass_utils import run_bass_kernel_spmd  # noqa: E402
from concourse.alu_op_type import AluOpType  # noqa: E402

F32 = mybir.dt.float32
F16 = mybir.dt.float16
AF = mybir.ActivationFunctionType

B = 131072
D = 768
P = 8
PD = 16
C = 5
CD = 16
NCORES = 8
BC = B // NCORES          # 16384 batch rows per core
NB = 512                  # batch columns per tile
NT = BC // NB             # 32 tiles

# fp16 constant blob column offsets
OWP = 0                   # [128, 768]   mm1 stationary (6 chunks of 128)
OWBD = 768                # [128, 640]   u_hat stationary, per class [128,128]
OPSEL = 1408              # [128, 400]   itr0 s80 stationary (0.2 folded)
OWFL = 1808               # [128, 400]   itr>0 s80 stationary (W per class)
OBSEL = 2208              # [40, 640]    cn -> (p,i) broadcast, per class
OASEL = 2848              # [128, 200]   agreement contraction, per class
OVBC = 3048               # [80, 640]    v80 -> (p,j) broadcast, per class
OSSEL = 3688              # [128, 8]     sum over (j) at fixed p
OSBC = 3696               # [8, 128]     broadcast p -> (p,j)
OJSEL = 3824              # [80, 8]      sum over j at fixed c
OGBC = 3832               # [8, 80]      broadcast c -> (c,j)
OCSEL = 3912              # [40, 8]      sum over c at fixed p  (logits (c,p))
OCBC = 3920               # [8, 40]      broadcast p -> (c,p)
OID80 = 3960              # [80, 80]     identity (v transposes)
CW = 4040


def build_consts(Wp, bp, W):
    """Host-side packing of all selectors + weights into one fp16 blob
    plus the fp32 bias column."""
    Wp = np.asarray(Wp, np.float32)
    bp = np.asarray(bp, np.float32)
    W = np.asarray(W, np.float32)

    cst = np.zeros((128, CW), np.float32)

    # mm1 stationary: wp[(d_sub), k*128 + (p,o)] = Wp[d, p, o] with d = k*128+d_sub
    wp_flat = Wp.transpose(1, 0, 2).reshape(D, 128)            # [d, (p,o)]
    cst[:, OWP:OWP + 768] = wp_flat.reshape(6, 128, 128).transpose(1, 0, 2).reshape(128, 768)

    # u_hat stationary (block-diag over p): wbd[(p,i), c*128+(p,j)] = W[p,c,i,j]
    for p in range(P):
        for c in range(C):
            cst[p * 16:(p + 1) * 16, OWBD + c * 128 + p * 16:OWBD + c * 128 + (p + 1) * 16] = W[p, c]

    # itr0 s: psel[(p,j), c*80 + (c',j')] = .2 * d_jj' * d_c'c
    for c in range(C):
        for p in range(P):
            for j in range(CD):
                cst[p * 16 + j, OPSEL + c * 80 + c * 16 + j] = 0.2

    # itr>0 s: wfl[(p,i), c*80 + (c', j)] = W[p,c,i,j] * d_c'c
    for c in range(C):
        for p in range(P):
            cst[p * 16:(p + 1) * 16, OWFL + c * 80 + c * 16:OWFL + c * 80 + (c + 1) * 16] = W[p, c]

    # bsel[(c'p'), c*128 + (p,i)] = d_c'c d_p'p
    for c in range(C):
        for p in range(P):
            cst[c * 8 + p, OBSEL + c * 128 + p * 16:OBSEL + c * 128 + (p + 1) * 16] = 1.0

    # asel[(p,j), c*40 + (c',p')] = d_pp' d_c'c
    for c in range(C):
        for p in range(P):
            for j in range(CD):
                cst[p * 16 + j, OASEL + c * 40 + c * 8 + p] = 1.0

    # vbc[(c'',j'), c*128 + (p,j)] = d_c''c d_j'j
    for c in range(C):
        for p in range(P):
            for j in range(CD):
                cst[c * 16 + j, OVBC + c * 128 + p * 16 + j] = 1.0

    # ssel[(p,j), p'] = d_pp'
    for p in range(P):
        for j in range(PD):
            cst[p * 16 + j, OSSEL + p] = 1.0

    # sbc[p', (p,j)]
    for p in range(P):
        cst[p, OSBC + p * 16:OSBC + (p + 1) * 16] = 1.0

    # jsel[(c,j), c']
    for c in range(C):
        for j in range(CD):
            cst[c * 16 + j, OJSEL + c] = 1.0

    # gbc[c', (c,j)]
    for c in range(C):
        cst[c, OGBC + c * 16:OGBC + (c + 1) * 16] = 1.0

    # csel[(c,p), p'] / cbc[p', (c,p)]
    for c in range(C):
        for p in range(P):
            cst[c * 8 + p, OCSEL + p] = 1.0
            cst[p, OCBC + c * 8 + p] = 1.0

    cst[:80, OID80:OID80 + 80] = np.eye(80)

    bp_h = np.ascontiguousarray(bp.reshape(128, 1), dtype=np.float32)
    return np.ascontiguousarray(cst.astype(np.float16)), bp_h


def prep_x(x_core):
    """[bc, 768] fp32 -> tile-image [nt*128, 3072] fp16 with
    img[t*128+p, k*512+c] = x[t*512+c, k*128+p] (pre-transposed)."""
    bc = x_core.shape[0]
    nt = bc // NB
    xi = x_core.reshape(nt, NB, 6, 128).transpose(0, 3, 2, 1)  # [t, p, k, c]
    return np.ascontiguousarray(xi.reshape(nt * 128, 6 * NB), dtype=np.float16)


def build_nc(nt: int = NT) -> bass.Bass:
    bc = nt * NB
    nc = bacc.Bacc(None)

    x_d = nc.declare_dram_parameter("xt", [nt * 128, 6 * NB], F16, isOutput=False)
    cb_d = nc.declare_dram_parameter("cstb", [128, CW], F16, isOutput=False)
    cf_d = nc.declare_dram_parameter("cstf", [128, 1], F32, isOutput=False)
    v_d = nc.declare_dram_parameter("vout", [bc, C * CD], F32, isOutput=True)

    with tile.TileContext(nc) as tc, nc.allow_low_precision(reason="fp16 compute"):
        with (
            tc.sbuf_pool(name="const", bufs=1) as cpool,
            tc.sbuf_pool(name="xin", bufs=3) as xpool,
            tc.sbuf_pool(name="mid", bufs=2) as mpool,
            tc.sbuf_pool(name="uh", bufs=2) as uhpool,
            tc.sbuf_pool(name="rt", bufs=2) as rtpool,
            tc.sbuf_pool(name="sm", bufs=3) as smpool,
            tc.sbuf_pool(name="vo", bufs=2) as vopool,
            tc.psum_pool(name="pbig", bufs=4) as pbig,
            tc.psum_pool(name="ps80", bufs=1) as ps80p,
            tc.psum_pool(name="plog", bufs=1) as plogp,
            tc.psum_pool(name="psm", bufs=2) as psmp,
        ):
            # ---- constants: one DMA each, staged through DVE ----
            cst0 = cpool.tile([128, CW], F16)
            nc.sync.dma_start(out=cst0[:], in_=cb_d[:])
            cst = cpool.tile([128, CW], F16)
            nc.vector.tensor_copy(cst[:], cst0[:])
            bp0 = cpool.tile([128, 1], F32)
            nc.sync.dma_start(out=bp0[:], in_=cf_d[:])
            bp_sb = cpool.tile([128, 1], F32)
            nc.vector.tensor_copy(bp_sb[:], bp0[:])

            for it in range(nt):
                xts = xpool.tile([128, 6 * NB], F16, tag="xin")
                nc.sync.dma_start(out=xts[:], in_=x_d[it * 128:(it + 1) * 128, :])

                # ---- mm1: u_pre[(p,o), b] = Wp^T x^T + bp ----
                pm = pbig.tile([128, NB], F32, tag="big")
                for k in range(6):
                    nc.tensor.matmul(
                        pm[:], cst[:, OWP + k * 128:OWP + (k + 1) * 128],
                        xts[:, k * NB:(k + 1) * NB],
                        start=(k == 0), stop=(k == 5))
                u_pre = mpool.tile([128, NB], F16, tag="upre")
                nc.scalar.activation(u_pre[:], pm[:], AF.Identity,
                                     bias=bp_sb[:], scale=1.0)
                usq = mpool.tile([128, NB], F16, tag="usq")
                nc.scalar.activation(usq[:], pm[:], AF.Square,
                                     bias=bp_sb[:], scale=1.0)

                # ---- squash factor f = exp(.5 ln q - ln(1+q)) ----
                pq = psmp.tile([8, NB], F32, tag="sm")
                nc.tensor.matmul(pq[:], cst[:, OSSEL:OSSEL + 8], usq[:],
                                 start=True, stop=True)
                lnq = smpool.tile([8, NB], F32, tag="lna")
                nc.scalar.activation(lnq[:], pq[:], AF.Ln)
                l1q = smpool.tile([8, NB], F32, tag="lnb")
                nc.scalar.activation(l1q[:], pq[:], AF.Ln, bias=1.0)
                z = smpool.tile([8, NB], F32, tag="zz")
                nc.vector.scalar_tensor_tensor(
                    z[:], lnq[:], 0.5, l1q[:],
                    op0=AluOpType.mult, op1=AluOpType.subtract)
                fz = smpool.tile([8, NB], F16, tag="ff")
                nc.scalar.activation(fz[:], z[:], AF.Exp)
                pfb = pbig.tile([128, NB], F32, tag="big")
                nc.tensor.matmul(pfb[:], cst[:8, OSBC:OSBC + 128], fz[:],
                                 start=True, stop=True)
                u = mpool.tile([128, NB], F16, tag="uu")
                nc.vector.tensor_mul(u[:], u_pre[:], pfb[:])

                # ---- u_hat per class ----
                uh = []
                for c in range(C):
                    puh = pbig.tile([128, NB], F32, tag="big")
                    nc.tensor.matmul(
                        puh[:], cst[:, OWBD + c * 128:OWBD + (c + 1) * 128],
                        u[:], start=True, stop=True)
                    uhc = uhpool.tile([128, NB], F16, tag=f"uh{c}")
                    if c in (1, 3):
                        nc.scalar.copy(uhc[:], puh[:])
                    else:
                        nc.vector.tensor_copy(uhc[:], puh[:])
                    uh.append(uhc)

                plg = plogp.tile([40, NB], F32, tag="lg")

                for itr in range(3):
                    if itr == 0:
                        ps = ps80p.tile([80, NB], F32, tag="ps")
                        for c in range(C):
                            nc.tensor.matmul(
                                ps[:], cst[:, OPSEL + c * 80:OPSEL + (c + 1) * 80],
                                uh[c][:], start=(c == 0), stop=(c == 4))
                    else:
                        # softmax over classes of logits [ (c,p), b ]
                        e = rtpool.tile([40, NB], F16, tag="ee")
                        nc.scalar.activation(e[:], plg[:], AF.Exp)
                        pden = psmp.tile([8, NB], F32, tag="sm")
                        nc.tensor.matmul(pden[:], cst[:40, OCSEL:OCSEL + 8],
                                         e[:], start=True, stop=True)
                        rdf = smpool.tile([8, NB], F32, tag="rdf")
                        nc.vector.reciprocal_approx_fast(out=rdf[:], in_=pden[:])
                        rdh = smpool.tile([8, NB], F16, tag="rdh")
                        nc.scalar.copy(rdh[:], rdf[:])
                        pdb = psmp.tile([40, NB], F32, tag="sm")
                        nc.tensor.matmul(pdb[:], cst[:8, OCBC:OCBC + 40],
                                         rdh[:], start=True, stop=True)
                        cn = rtpool.tile([40, NB], F16, tag="cn")
                        nc.vector.tensor_mul(cn[:], e[:], pdb[:])

                        ps = ps80p.tile([80, NB], F32, tag="ps")
                        for c in range(C):
                            pcb = pbig.tile([128, NB], F32, tag="big")
                            nc.tensor.matmul(
                                pcb[:], cst[:40, OBSEL + c * 128:OBSEL + (c + 1) * 128],
                                cn[:], start=True, stop=True)
                            tcm = rtpool.tile([128, NB], F16, tag=f"t{c}")
                            nc.vector.tensor_mul(tcm[:], u[:], pcb[:])
                            nc.tensor.matmul(
                                ps[:], cst[:, OWFL + c * 80:OWFL + (c + 1) * 80],
                                tcm[:], start=(c == 0), stop=(c == 4))

                    # ---- g = squash factor of s (per class) ----
                    ssq = rtpool.tile([80, NB], F16, tag="ssq")
                    nc.scalar.activation(ssq[:], ps[:], AF.Square)
                    pvq = psmp.tile([8, NB], F32, tag="sm")
                    nc.tensor.matmul(pvq[:], cst[:80, OJSEL:OJSEL + 8],
                                     ssq[:], start=True, stop=True)
                    lnv = smpool.tile([8, NB], F32, tag="lna")
                    nc.scalar.activation(lnv[:], pvq[:], AF.Ln)
                    l1v = smpool.tile([8, NB], F32, tag="lnb")
                    nc.scalar.activation(l1v[:], pvq[:], AF.Ln, bias=1.0)
                    zv = smpool.tile([8, NB], F32, tag="zz")
                    nc.vector.scalar_tensor_tensor(
                        zv[:], lnv[:], 0.5, l1v[:],
                        op0=AluOpType.mult, op1=AluOpType.subtract)
                    g = smpool.tile([8, NB], F16, tag="gg")
                    nc.scalar.activation(g[:], zv[:], AF.Exp)
                    pgb = psmp.tile([80, NB], F32, tag="sm")
                    nc.tensor.matmul(pgb[:], cst[:8, OGBC:OGBC + 80], g[:],
                                     start=True, stop=True)
                    gb = rtpool.tile([80, NB], F16, tag="gb")
                    nc.scalar.copy(gb[:], pgb[:])
                    v80 = rtpool.tile([80, NB], F16, tag="v80")
                    nc.vector.tensor_mul(v80[:], gb[:], ps[:])

                    if itr < 2:
                        # logits += sum_j uh*v  (accumulated in PSUM)
                        for c in range(C):
                            pvb = pbig.tile([128, NB], F32, tag="big")
                            nc.tensor.matmul(
                                pvb[:], cst[:80, OVBC + c * 128:OVBC + (c + 1) * 128],
                                v80[:], start=True, stop=True)
                            pr = rtpool.tile([128, NB], F16, tag=f"pr{c}")
                            nc.vector.tensor_mul(pr[:], uh[c][:], pvb[:])
                            nc.tensor.matmul(
                                plg[:], cst[:, OASEL + c * 40:OASEL + (c + 1) * 40],
                                pr[:], start=(itr == 0 and c == 0),
                                stop=(c == 4),
                                skip_group_check=(itr == 1))
                    else:
                        vo = vopool.tile([128, 4, 80], F32, tag="vo")
                        for q in range(4):
                            pvt = pbig.tile([128, 80], F16, tag="big")
                            nc.tensor.transpose(
                                pvt[:], v80[:, q * 128:(q + 1) * 128],
                                cst[:80, OID80:OID80 + 80])
                            nc.scalar.copy(vo[:, q, :], pvt[:])
                        dst = v_d[it * NB:(it + 1) * NB, :].rearrange(
                            "(q p) j -> p q j", p=128)
                        nc.sync.dma_start(out=dst, in_=vo[:])

    nc.compile()
    return nc


_NC_CACHE: dict = {}


def _get_nc(nt: int) -> bass.Bass:
    if nt not in _NC_CACHE:
        _NC_CACHE[nt] = build_nc(nt)
    return _NC_CACHE[nt]


def make_in_maps(x, Wp, bp, W, nt: int = NT):
    """Shard + host-prep inputs for the SPMD launch (nt tiles per core)."""
    x = np.asarray(x, np.float32)
    cstb, cstf = build_consts(Wp, bp, W)
    bc = nt * NB
    maps = []
    for i in range(NCORES):
        xc = x[i * bc:(i + 1) * bc] if nt == NT else x[i * bc:(i + 1) * bc]
        maps.append({"xt": prep_x(xc), "cstb": cstb, "cstf": cstf})
    return maps


def kernel(x, Wp, bp, W):
    nc = _get_nc(NT)
    in_maps = make_in_maps(x, Wp, bp, W, NT)
    res = run_bass_kernel_spmd(nc, in_maps, list(range(NCORES)))
    out = np.concatenate([res.results[i]["vout"] for i in range(NCORES)], axis=0)
    return out.reshape(B, C, CD)


# revision 5
# speedup vs baseline: 1.5124x; 1.0177x over previous
"""Trainium2 Bass kernel for CapsuleLayer (nn_CapsuleLayer_45552423142009).

Computes, for x[B,768]:
  u = squash(x @ Wp + bp)            # [B, 8, 16]  (squash over last dim)
  u_hat[b,p,c,:] = u[b,p,:] @ W[p,c] # [B, 8, 5, 16]
  3 iterations of dynamic routing -> v [B, 5, 16]

Strategy: pure data-parallel over 8 NeuronCores (batch sharded 16384/core).
On-chip layout is "transposed": features on partitions, batch on the free
dim (512-wide tiles).  Key differences vs the v1 kernel:
  - x is transposed + cast to fp16 on the host, so the device does no
    PE transposes of x and reads half the HBM bytes.
  - all matmul operands are fp16 (1 cycle/row on the PE vs ~2 for f32r).
  - squash factors via exp/ln only (f = sqrt(q)/(1+q) = exp(.5*ln q -
    ln(1+q))), so the ACT engine stays on one table set (no ~2.7us
    ACT_TABLE_LOAD thrash), and softmax recip uses the fast DVE approx.
  - s/v are packed [80=(c,j), b] via M=80 accumulating selector matmuls:
    one vsq / g-broadcast / v-multiply per iteration instead of five.
  - routing logits accumulate directly in one pinned PSUM bank across
    iterations (PE start=False accumulation), read in place by Exp.
"""

import sys
import numpy as np

sys.path.insert(0, "/opt/trn_rl_repo")

from concourse import bass, bacc, mybir  # noqa: E402
from concourse import tile  # noqa: E402
from concourse.bass_utils import run_bass_kernel_spmd  # noqa: E402
from concourse.alu_op_type import AluOpType  # noqa: E402

F32 = mybir.dt.float32
F16 = mybir.dt.float16
AF = mybir.ActivationFunctionType

B = 131072
D = 768
P = 8
PD = 16
C = 5
CD = 16
NCORES = 8
BC = B // NCORES          # 16384 batch rows per core
NB = 512                  # batch columns per tile
NT = BC // NB             # 32 tiles

# fp16 constant blob column offsets
OWP = 0                   # [128, 768]   mm1 stationary (6 chunks of 128)
OWBD = 768                # [128, 640]   u_hat stationary, per class [128,128]
OPSEL = 1408              # [128, 400]   itr0 s80 stationary (0.2 folded)
OWFL = 1808               # [128, 400]   itr>0 s80 stationary (W per class)
OBSEL = 2208              # [40, 640]    cn -> (p,i) broadcast, per class
OASEL = 2848              # [128, 200]   agreement contraction, per class
OVBC = 3048               # [80, 640]    v80 -> (p,j) broadcast, per class
OSSEL = 3688              # [128, 8]     sum over (j) at fixed p
OSBC = 3696               # [8, 128]     broadcast p -> (p,j)
OJSEL = 3824              # [80, 8]      sum over j at fixed c
OGBC = 3832               # [8, 80]      broadcast c -> (c,j)
OCSEL = 3912              # [40, 8]      sum over c at fixed p  (logits (c,p))
OCBC = 3920               # [8, 40]      broadcast p -> (c,p)
OID80 = 3960              # [80, 80]     identity (v transposes)
CW = 4040


def build_consts(Wp, bp, W):
    """Host-side packing of all selectors + weights into one fp16 blob
    plus the fp32 bias column."""
    Wp = np.asarray(Wp, np.float32)
    bp = np.asarray(bp, np.float32)
    W = np.asarray(W, np.float32)

    cst = np.zeros((128, CW), np.float32)

    # mm1 stationary: wp[(d_sub), k*128 + (p,o)] = Wp[d, p, o] with d = k*128+d_sub
    wp_flat = Wp.transpose(1, 0, 2).reshape(D, 128)            # [d, (p,o)]
    cst[:, OWP:OWP + 768] = wp_flat.reshape(6, 128, 128).transpose(1, 0, 2).reshape(128, 768)

    # u_hat stationary (block-diag over p): wbd[(p,i), c*128+(p,j)] = W[p,c,i,j]
    for p in range(P):
        for c in range(C):
            cst[p * 16:(p + 1) * 16, OWBD + c * 128 + p * 16:OWBD + c * 128 + (p + 1) * 16] = W[p, c]

    # itr0 s: psel[(p,j), c*80 + (c',j')] = .2 * d_jj' * d_c'c
    for c in range(C):
        for p in range(P):
            for j in range(CD):
                cst[p * 16 + j, OPSEL + c * 80 + c * 16 + j] = 0.2

    # itr>0 s: wfl[(p,i), c*80 + (c', j)] = W[p,c,i,j] * d_c'c
    for c in range(C):
        for p in range(P):
            cst[p * 16:(p + 1) * 16, OWFL + c * 80 + c * 16:OWFL + c * 80 + (c + 1) * 16] = W[p, c]

    # bsel[(c'p'), c*128 + (p,i)] = d_c'c d_p'p
    for c in range(C):
        for p in range(P):
            cst[c * 8 + p, OBSEL + c * 128 + p * 16:OBSEL + c * 128 + (p + 1) * 16] = 1.0

    # asel[(p,j), c*40 + (c',p')] = d_pp' d_c'c
    for c in range(C):
        for p in range(P):
            for j in range(CD):
                cst[p * 16 + j, OASEL + c * 40 + c * 8 + p] = 1.0

    # vbc[(c'',j'), c*128 + (p,j)] = d_c''c d_j'j
    for c in range(C):
        for p in range(P):
            for j in range(CD):
                cst[c * 16 + j, OVBC + c * 128 + p * 16 + j] = 1.0

    # ssel[(p,j), p'] = d_pp'
    for p in range(P):
        for j in range(PD):
            cst[p * 16 + j, OSSEL + p] = 1.0

    # sbc[p', (p,j)]
    for p in range(P):
        cst[p, OSBC + p * 16:OSBC + (p + 1) * 16] = 1.0

    # jsel[(c,j), c']
    for c in range(C):
        for j in range(CD):
            cst[c * 16 + j, OJSEL + c] = 1.0

    # gbc[c', (c,j)]
    for c in range(C):
        cst[c, OGBC + c * 16:OGBC + (c + 1) * 16] = 1.0

    # csel[(c,p), p'] / cbc[p', (c,p)]
    for c in range(C):
        for p in range(P):
            cst[c * 8 + p, OCSEL + p] = 1.0
            cst[p, OCBC + c * 8 + p] = 1.0

    cst[:80, OID80:OID80 + 80] = np.eye(80)

    bp_h = np.ascontiguousarray(bp.reshape(128, 1), dtype=np.float32)
    return np.ascontiguousarray(cst.astype(np.float16)), bp_h


def prep_x(x_core):
    """[bc, 768] fp32 -> tile-image [nt*128, 3072] fp16 with
    img[t*128+p, k*512+c] = x[t*512+c, k*128+p] (pre-transposed)."""
    bc = x_core.shape[0]
    nt = bc // NB
    xi = x_core.reshape(nt, NB, 6, 128).transpose(0, 3, 2, 1)  # [t, p, k, c]
    return np.ascontiguousarray(xi.reshape(nt * 128, 6 * NB), dtype=np.float16)


def build_nc(nt: int = NT) -> bass.Bass:
    bc = nt * NB
    nc = bacc.Bacc(None)

    x_d = nc.declare_dram_parameter("xt", [nt * 128, 6 * NB], F16, isOutput=False)
    cb_d = nc.declare_dram_parameter("cstb", [128, CW], F16, isOutput=False)
    cf_d = nc.declare_dram_parameter("cstf", [128, 1], F32, isOutput=False)
    v_d = nc.declare_dram_parameter("vout", [bc, C * CD], F32, isOutput=True)

    with tile.TileContext(nc) as tc, nc.allow_low_precision(reason="fp16 compute"):
        with (
            tc.sbuf_pool(name="const", bufs=1) as cpool,
            tc.sbuf_pool(name="xin", bufs=3) as xpool,
            tc.sbuf_pool(name="mid", bufs=2) as mpool,
            tc.sbuf_pool(name="uh", bufs=2) as uhpool,
            tc.sbuf_pool(name="rt", bufs=2) as rtpool,
            tc.sbuf_pool(name="sm", bufs=3) as smpool,
            tc.sbuf_pool(name="vo", bufs=2) as vopool,
            tc.psum_pool(name="pbig", bufs=3) as pbig,
            tc.psum_pool(name="ps80", bufs=2) as ps80p,
            tc.psum_pool(name="psm", bufs=3) as psmp,
        ):
            # ---- pin the one ACT table set holding Ln+Exp+Square+Copy+
            # Identity, so the first-fit table chooser never thrashes ----
            nc.scalar.add_instruction(mybir.InstLoadActFuncSet(
                name=nc.get_next_instruction_name(), ins=[], outs=[],
                act_func_set_id=6))  # natural_log_exp_and_others

            # ---- constants: one DMA each, staged through DVE ----
            cst0 = cpool.tile([128, CW], F16)
            nc.sync.dma_start(out=cst0[:], in_=cb_d[:])
            cst = cpool.tile([128, CW], F16)
            nc.vector.tensor_copy(cst[:], cst0[:])
            bp0 = cpool.tile([128, 1], F32)
            nc.sync.dma_start(out=bp0[:], in_=cf_d[:])
            bp_sb = cpool.tile([128, 1], F32)
            nc.vector.tensor_copy(bp_sb[:], bp0[:])

            for it in range(nt):
                xts = xpool.tile([128, 6 * NB], F16, tag="xin")
                nc.sync.dma_start(out=xts[:], in_=x_d[it * 128:(it + 1) * 128, :])

                # ---- mm1: u_pre[(p,o), b] = Wp^T x^T + bp ----
                pm = pbig.tile([128, NB], F32, tag="big")
                for k in range(6):
                    nc.tensor.matmul(
                        pm[:], cst[:, OWP + k * 128:OWP + (k + 1) * 128],
                        xts[:, k * NB:(k + 1) * NB],
                        start=(k == 0), stop=(k == 5))
                u_pre = mpool.tile([128, NB], F16, tag="upre")
                nc.scalar.activation(u_pre[:], pm[:], AF.Identity,
                                     bias=bp_sb[:], scale=1.0)
                usq = mpool.tile([128, NB], F16, tag="usq")
                nc.scalar.activation(usq[:], pm[:], AF.Square,
                                     bias=bp_sb[:], scale=1.0)

                # ---- squash factor f = exp(.5 ln q - ln(1+q)) ----
                pq = psmp.tile([8, NB], F32, tag="sm")
                nc.tensor.matmul(pq[:], cst[:, OSSEL:OSSEL + 8], usq[:],
                                 start=True, stop=True)
                lnq = smpool.tile([8, NB], F32, tag="lna")
                nc.scalar.activation(lnq[:], pq[:], AF.Ln)
                l1q = smpool.tile([8, NB], F32, tag="lnb")
                nc.scalar.activation(l1q[:], pq[:], AF.Ln, bias=1.0)
                z = smpool.tile([8, NB], F32, tag="zz")
                nc.vector.scalar_tensor_tensor(
                    z[:], lnq[:], 0.5, l1q[:],
                    op0=AluOpType.mult, op1=AluOpType.subtract)
                fz = smpool.tile([8, NB], F16, tag="ff")
                nc.scalar.activation(fz[:], z[:], AF.Exp)
                pfb = pbig.tile([128, NB], F32, tag="big")
                nc.tensor.matmul(pfb[:], cst[:8, OSBC:OSBC + 128], fz[:],
                                 start=True, stop=True)
                u = mpool.tile([128, NB], F16, tag="uu")
                nc.vector.tensor_mul(u[:], u_pre[:], pfb[:])

                # ---- u_hat per class ----
                uh = []
                for c in range(C):
                    puh = pbig.tile([128, NB], F32, tag="big")
                    nc.tensor.matmul(
                        puh[:], cst[:, OWBD + c * 128:OWBD + (c + 1) * 128],
                        u[:], start=True, stop=True)
                    uhc = uhpool.tile([128, NB], F16, tag=f"uh{c}")
                    if c in (1, 3):
                        nc.scalar.copy(uhc[:], puh[:])
                    else:
                        nc.vector.tensor_copy(uhc[:], puh[:])
                    uh.append(uhc)

                lg_sb = None

                for itr in range(3):
                    if itr == 0:
                        ps = ps80p.tile([80, NB], F32, tag="ps")
                        for c in range(C):
                            nc.tensor.matmul(
                                ps[:], cst[:, OPSEL + c * 80:OPSEL + (c + 1) * 80],
                                uh[c][:], start=(c == 0), stop=(c == 4))
                    else:
                        # softmax over classes of logits [ (c,p), b ]
                        e = rtpool.tile([40, NB], F16, tag="ee")
                        nc.scalar.activation(e[:], lg_sb[:], AF.Exp)
                        pden = psmp.tile([8, NB], F32, tag="sm")
                        nc.tensor.matmul(pden[:], cst[:40, OCSEL:OCSEL + 8],
                                         e[:], start=True, stop=True)
                        rdf = smpool.tile([8, NB], F32, tag="rdf")
                        nc.vector.reciprocal_approx_fast(out=rdf[:], in_=pden[:])
                        rdh = smpool.tile([8, NB], F16, tag="rdh")
                        nc.scalar.copy(rdh[:], rdf[:])
                        pdb = psmp.tile([40, NB], F32, tag="sm")
                        nc.tensor.matmul(pdb[:], cst[:8, OCBC:OCBC + 40],
                                         rdh[:], start=True, stop=True)
                        cn = rtpool.tile([40, NB], F16, tag="cn")
                        nc.vector.tensor_mul(cn[:], e[:], pdb[:])

                        ps = ps80p.tile([80, NB], F32, tag="ps")
                        for c in range(C):
                            pcb = pbig.tile([128, NB], F32, tag="big")
                            nc.tensor.matmul(
                                pcb[:], cst[:40, OBSEL + c * 128:OBSEL + (c + 1) * 128],
                                cn[:], start=True, stop=True)
                            tcm = rtpool.tile([128, NB], F16, tag=f"t{c}")
                            nc.vector.tensor_mul(tcm[:], u[:], pcb[:])
                            nc.tensor.matmul(
                                ps[:], cst[:, OWFL + c * 80:OWFL + (c + 1) * 80],
                                tcm[:], start=(c == 0), stop=(c == 4))

                    # ---- g = squash factor of s (per class) ----
                    ssq = rtpool.tile([80, NB], F16, tag="ssq")
                    nc.scalar.activation(ssq[:], ps[:], AF.Square)
                    pvq = psmp.tile([8, NB], F32, tag="sm")
                    nc.tensor.matmul(pvq[:], cst[:80, OJSEL:OJSEL + 8],
                                     ssq[:], start=True, stop=True)
                    lnv = smpool.tile([8, NB], F32, tag="lna")
                    nc.scalar.activation(lnv[:], pvq[:], AF.Ln)
                    l1v = smpool.tile([8, NB], F32, tag="lnb")
                    nc.scalar.activation(l1v[:], pvq[:], AF.Ln, bias=1.0)
                    zv = smpool.tile([8, NB], F32, tag="zz")
                    nc.vector.scalar_tensor_tensor(
                        zv[:], lnv[:], 0.5, l1v[:],
                        op0=AluOpType.mult, op1=AluOpType.subtract)
                    g = smpool.tile([8, NB], F16, tag="gg")
                    nc.scalar.activation(g[:], zv[:], AF.Exp)
                    pgb = psmp.tile([80, NB], F32, tag="sm")
                    nc.tensor.matmul(pgb[:], cst[:8, OGBC:OGBC + 80], g[:],
                                     start=True, stop=True)
                    gb = rtpool.tile([80, NB], F16, tag="gb")
                    nc.scalar.copy(gb[:], pgb[:])
                    v80 = rtpool.tile([80, NB], F16, tag="v80")
                    nc.vector.tensor_mul(v80[:], gb[:], ps[:])

                    if itr < 2:
                        # logits += sum_j uh*v
                        pat = psmp.tile([40, NB], F32, tag="sm")
                        for c in range(C):
                            pvb = pbig.tile([128, NB], F32, tag="big")
                            nc.tensor.matmul(
                                pvb[:], cst[:80, OVBC + c * 128:OVBC + (c + 1) * 128],
                                v80[:], start=True, stop=True)
                            pr = rtpool.tile([128, NB], F16, tag=f"pr{c}")
                            nc.vector.tensor_mul(pr[:], uh[c][:], pvb[:])
                            nc.tensor.matmul(
                                pat[:], cst[:, OASEL + c * 40:OASEL + (c + 1) * 40],
                                pr[:], start=(c == 0), stop=(c == 4))
                        if itr == 0:
                            lg_sb = rtpool.tile([40, NB], F32, tag="lg")
                            nc.vector.tensor_copy(lg_sb[:], pat[:])
                        else:
                            lg2 = rtpool.tile([40, NB], F32, tag="lg2")
                            nc.vector.tensor_add(lg2[:], lg_sb[:], pat[:])
                            lg_sb = lg2
                    else:
                        vo = vopool.tile([128, 4, 80], F32, tag="vo")
                        for q in range(4):
                            pvt = pbig.tile([128, 80], F16, tag="big")
                            nc.tensor.transpose(
                                pvt[:], v80[:, q * 128:(q + 1) * 128],
                                cst[:80, OID80:OID80 + 80])
                            nc.scalar.copy(vo[:, q, :], pvt[:])
                        dst = v_d[it * NB:(it + 1) * NB, :].rearrange(
                            "(q p) j -> p q j", p=128)
                        nc.sync.dma_start(out=dst, in_=vo[:])

    nc.compile()
    return nc


_NC_CACHE: dict = {}


def _get_nc(nt: int) -> bass.Bass:
    if nt not in _NC_CACHE:
        _NC_CACHE[nt] = build_nc(nt)
    return _NC_CACHE[nt]


def make_in_maps(x, Wp, bp, W, nt: int = NT):
    """Shard + host-prep inputs for the SPMD launch (nt tiles per core)."""
    x = np.asarray(x, np.float32)
    cstb, cstf = build_consts(Wp, bp, W)
    bc = nt * NB
    maps = []
    for i in range(NCORES):
        xc = x[i * bc:(i + 1) * bc] if nt == NT else x[i * bc:(i + 1) * bc]
        maps.append({"xt": prep_x(xc), "cstb": cstb, "cstf": cstf})
    return maps


def kernel(x, Wp, bp, W):
    nc = _get_nc(NT)
    in_maps = make_in_maps(x, Wp, bp, W, NT)
    res = run_bass_kernel_spmd(nc, in_maps, list(range(NCORES)))
    out = np.concatenate([res.results[i]["vout"] for i in range(NCORES)], axis=0)
    return out.reshape(B, C, CD)


# revision 9
# speedup vs baseline: 1.5968x; 1.0558x over previous
"""Trainium2 Bass kernel for CapsuleLayer (nn_CapsuleLayer_45552423142009).

Computes, for x[B,768]:
  u = squash(x @ Wp + bp)            # [B, 8, 16]  (squash over last dim)
  u_hat[b,p,c,:] = u[b,p,:] @ W[p,c] # [B, 8, 5, 16]
  3 iterations of dynamic routing -> v [B, 5, 16]

Strategy: pure data-parallel over 8 NeuronCores (batch sharded 16384/core).
On-chip layout is "transposed": features on partitions, batch on the free
dim (512-wide tiles).  Key differences vs the v1 kernel:
  - x is transposed + cast to fp16 on the host, so the device does no
    PE transposes of x and reads half the HBM bytes.
  - all matmul operands are fp16 (1 cycle/row on the PE vs ~2 for f32r).
  - squash factors via exp/ln only (f = sqrt(q)/(1+q) = exp(.5*ln q -
    ln(1+q))), so the ACT engine stays on one table set (no ~2.7us
    ACT_TABLE_LOAD thrash), and softmax recip uses the fast DVE approx.
  - s/v are packed [80=(c,j), b] via M=80 accumulating selector matmuls:
    one vsq / g-broadcast / v-multiply per iteration instead of five.
  - routing logits accumulate directly in one pinned PSUM bank across
    iterations (PE start=False accumulation), read in place by Exp.
"""

import sys
import numpy as np

sys.path.insert(0, "/opt/trn_rl_repo")

from concourse import bass, bacc, mybir  # noqa: E402
from concourse import tile  # noqa: E402
from concourse.bass_utils import run_bass_kernel_spmd  # noqa: E402
from concourse.alu_op_type import AluOpType  # noqa: E402

F32 = mybir.dt.float32
F16 = mybir.dt.float16
AF = mybir.ActivationFunctionType

B = 131072
D = 768
P = 8
PD = 16
C = 5
CD = 16
NCORES = 8
BC = B // NCORES          # 16384 batch rows per core
NB = 512                  # batch columns per tile
NT = BC // NB             # 32 tiles

# fp16 constant blob column offsets
OWP = 0                   # [128, 768]   mm1 stationary (6 chunks of 128)
OWBD = 768                # [128, 640]   u_hat stationary, per class [128,128]
OS0 = 1408                # [128, 80]    itr0 s80 stationary (0.2*W, all classes)
OWFL = 1808               # [128, 400]   itr>0 s80 stationary (W per class)
OBSEL = 2208              # [40, 640]    cn -> (p,i) broadcast, per class
OASEL = 2848              # [128, 200]   agreement contraction, per class
OVBC = 3048               # [80, 640]    v80 -> (p,j) broadcast, per class
OSSEL = 3688              # [128, 8]     sum over (j) at fixed p
OSBC = 3696               # [8, 128]     broadcast p -> (p,j)
OJSEL = 3824              # [80, 8]      sum over j at fixed c
OGBC = 3832               # [8, 80]      broadcast c -> (c,j)
OCSEL = 3912              # [40, 8]      sum over c at fixed p  (logits (c,p))
OCBC = 3920               # [8, 40]      broadcast p -> (c,p)
OID80 = 3960              # [80, 80]     identity (v transposes)
CW = 4040


def build_consts(Wp, bp, W):
    """Host-side packing of all selectors + weights into one fp16 blob
    plus the fp32 bias column."""
    Wp = np.asarray(Wp, np.float32)
    bp = np.asarray(bp, np.float32)
    W = np.asarray(W, np.float32)

    cst = np.zeros((128, CW), np.float32)

    # mm1 stationary: wp[(d_sub), k*128 + (p,o)] = Wp[d, p, o] with d = k*128+d_sub
    wp_flat = Wp.transpose(1, 0, 2).reshape(D, 128)            # [d, (p,o)]
    cst[:, OWP:OWP + 768] = wp_flat.reshape(6, 128, 128).transpose(1, 0, 2).reshape(128, 768)

    # u_hat stationary (block-diag over p): wbd[(p,i), c*128+(p,j)] = W[p,c,i,j]
    for p in range(P):
        for c in range(C):
            cst[p * 16:(p + 1) * 16, OWBD + c * 128 + p * 16:OWBD + c * 128 + (p + 1) * 16] = W[p, c]

    # itr0 s in one matmul: s0[(c,j)] = .2 sum_p u_p @ W_pc
    # os0[(p,i), c*16+j] = .2 * W[p,c,i,j]
    for c in range(C):
        for p in range(P):
            cst[p * 16:(p + 1) * 16, OS0 + c * 16:OS0 + (c + 1) * 16] += 0.2 * W[p, c]

    # itr>0 s: wfl[(p,i), c*80 + (c', j)] = W[p,c,i,j] * d_c'c
    for c in range(C):
        for p in range(P):
            cst[p * 16:(p + 1) * 16, OWFL + c * 80 + c * 16:OWFL + c * 80 + (c + 1) * 16] = W[p, c]

    # bsel[(c'p'), c*128 + (p,i)] = d_c'c d_p'p
    for c in range(C):
        for p in range(P):
            cst[c * 8 + p, OBSEL + c * 128 + p * 16:OBSEL + c * 128 + (p + 1) * 16] = 1.0

    # asel[(p,j), c*40 + (c',p')] = d_pp' d_c'c
    for c in range(C):
        for p in range(P):
            for j in range(CD):
                cst[p * 16 + j, OASEL + c * 40 + c * 8 + p] = 1.0

    # vbc[(c'',j'), c*128 + (p,j)] = d_c''c d_j'j
    for c in range(C):
        for p in range(P):
            for j in range(CD):
                cst[c * 16 + j, OVBC + c * 128 + p * 16 + j] = 1.0

    # ssel[(p,j), p'] = d_pp'
    for p in range(P):
        for j in range(PD):
            cst[p * 16 + j, OSSEL + p] = 1.0

    # sbc[p', (p,j)]
    for p in range(P):
        cst[p, OSBC + p * 16:OSBC + (p + 1) * 16] = 1.0

    # jsel[(c,j), c']
    for c in range(C):
        for j in range(CD):
            cst[c * 16 + j, OJSEL + c] = 1.0

    # gbc[c', (c,j)]
    for c in range(C):
        cst[c, OGBC + c * 16:OGBC + (c + 1) * 16] = 1.0

    # csel[(c,p), p'] / cbc[p', (c,p)]
    for c in range(C):
        for p in range(P):
            cst[c * 8 + p, OCSEL + p] = 1.0
            cst[p, OCBC + c * 8 + p] = 1.0

    cst[:80, OID80:OID80 + 80] = np.eye(80)

    bp_h = np.ascontiguousarray(bp.reshape(128, 1), dtype=np.float32)
    return np.ascontiguousarray(cst.astype(np.float16)), bp_h


def prep_x(x_core):
    """[bc, 768] fp32 -> tile-image [nt*128, 3072] fp16 with
    img[t*128+p, k*512+c] = x[t*512+c, k*128+p] (pre-transposed)."""
    bc = x_core.shape[0]
    nt = bc // NB
    xi = x_core.reshape(nt, NB, 6, 128).transpose(0, 3, 2, 1)  # [t, p, k, c]
    return np.ascontiguousarray(xi.reshape(nt * 128, 6 * NB), dtype=np.float16)


def build_nc(nt: int = NT) -> bass.Bass:
    bc = nt * NB
    nc = bacc.Bacc(None)

    x_d = nc.declare_dram_parameter("xt", [nt * 128, 6 * NB], F16, isOutput=False)
    cb_d = nc.declare_dram_parameter("cstb", [128, CW], F16, isOutput=False)
    cf_d = nc.declare_dram_parameter("cstf", [128, 1], F32, isOutput=False)
    v_d = nc.declare_dram_parameter("vout", [bc, C * CD], F32, isOutput=True)

    with tile.TileContext(nc) as tc, nc.allow_low_precision(reason="fp16 compute"):
        with (
            tc.sbuf_pool(name="const", bufs=1) as cpool,
            tc.sbuf_pool(name="xin", bufs=3) as xpool,
            tc.sbuf_pool(name="mid", bufs=2) as mpool,
            tc.sbuf_pool(name="uh", bufs=2) as uhpool,
            tc.sbuf_pool(name="rt", bufs=2) as rtpool,
            tc.sbuf_pool(name="sm", bufs=3) as smpool,
            tc.sbuf_pool(name="vo", bufs=2) as vopool,
            tc.psum_pool(name="pfront", bufs=2) as pfr,
            tc.psum_pool(name="pbcast", bufs=2) as pbc,
            tc.psum_pool(name="ps80", bufs=2) as ps80p,
            tc.psum_pool(name="psm", bufs=2) as psmp,
        ):
            # ---- pin the one ACT table set holding Ln+Exp+Square+Copy+
            # Identity, so the first-fit table chooser never thrashes ----
            nc.scalar.add_instruction(mybir.InstLoadActFuncSet(
                name=nc.get_next_instruction_name(), ins=[], outs=[],
                act_func_set_id=6))  # natural_log_exp_and_others

            # ---- constants: one DMA each, staged through DVE ----
            cst0 = cpool.tile([128, CW], F16)
            nc.sync.dma_start(out=cst0[:], in_=cb_d[:])
            cst = cpool.tile([128, CW], F16)
            nc.vector.tensor_copy(cst[:], cst0[:])
            bp0 = cpool.tile([128, 1], F32)
            nc.sync.dma_start(out=bp0[:], in_=cf_d[:])
            bp_sb = cpool.tile([128, 1], F32)
            nc.vector.tensor_copy(bp_sb[:], bp0[:])

            for it in range(nt):
                xts = xpool.tile([128, 6 * NB], F16, tag="xin")
                nc.sync.dma_start(out=xts[:], in_=x_d[it * 128:(it + 1) * 128, :])

                # ---- mm1: u_pre[(p,o), b] = Wp^T x^T + bp ----
                pm = pfr.tile([128, NB], F32, tag="fr")
                for k in range(6):
                    nc.tensor.matmul(
                        pm[:], cst[:, OWP + k * 128:OWP + (k + 1) * 128],
                        xts[:, k * NB:(k + 1) * NB],
                        start=(k == 0), stop=(k == 5))
                u_pre = mpool.tile([128, NB], F16, tag="upre")
                nc.scalar.activation(u_pre[:], pm[:], AF.Identity,
                                     bias=bp_sb[:], scale=1.0)
                usq = mpool.tile([128, NB], F16, tag="usq")
                nc.scalar.activation(usq[:], pm[:], AF.Square,
                                     bias=bp_sb[:], scale=1.0)

                # ---- squash factor f = exp(.5 ln q - ln(1+q)) ----
                pq = psmp.tile([8, NB], F32, tag="sm")
                nc.tensor.matmul(pq[:], cst[:, OSSEL:OSSEL + 8], usq[:],
                                 start=True, stop=True)
                lnq = smpool.tile([8, NB], F32, tag="lna")
                nc.scalar.activation(lnq[:], pq[:], AF.Ln)
                l1q = smpool.tile([8, NB], F32, tag="lnb")
                nc.scalar.activation(l1q[:], pq[:], AF.Ln, bias=1.0)
                z = smpool.tile([8, NB], F32, tag="zz")
                nc.vector.scalar_tensor_tensor(
                    z[:], lnq[:], 0.5, l1q[:],
                    op0=AluOpType.mult, op1=AluOpType.subtract)
                fz = smpool.tile([8, NB], F16, tag="ff")
                nc.scalar.activation(fz[:], z[:], AF.Exp)
                pfb = pfr.tile([128, NB], F32, tag="fr")
                nc.tensor.matmul(pfb[:], cst[:8, OSBC:OSBC + 128], fz[:],
                                 start=True, stop=True)
                u = mpool.tile([128, NB], F16, tag="uu")
                nc.vector.tensor_mul(u[:], u_pre[:], pfb[:])

                # ---- u_hat per class ----
                uh = []
                for c in range(C):
                    puh = pfr.tile([128, NB], F32, tag="fr")
                    nc.tensor.matmul(
                        puh[:], cst[:, OWBD + c * 128:OWBD + (c + 1) * 128],
                        u[:], start=True, stop=True)
                    uhc = uhpool.tile([128, NB], F16, tag=f"uh{c}")
                    if c in (0, 1, 3):
                        nc.scalar.copy(uhc[:], puh[:])
                    else:
                        nc.vector.tensor_copy(uhc[:], puh[:])
                    uh.append(uhc)

                lg_sb = None

                for itr in range(3):
                    if itr == 0:
                        ps = ps80p.tile([80, NB], F32, tag="ps")
                        nc.tensor.matmul(ps[:], cst[:, OS0:OS0 + 80], u[:],
                                         start=True, stop=True)
                    else:
                        # softmax over classes of logits [ (c,p), b ]
                        e = rtpool.tile([40, NB], F16, tag="ee")
                        nc.scalar.activation(e[:], lg_sb[:], AF.Exp)
                        pden = psmp.tile([8, NB], F32, tag="sm")
                        nc.tensor.matmul(pden[:], cst[:40, OCSEL:OCSEL + 8],
                                         e[:], start=True, stop=True)
                        rdf = smpool.tile([8, NB], F32, tag="rdf")
                        nc.vector.reciprocal_approx_fast(out=rdf[:], in_=pden[:])
                        rdh = smpool.tile([8, NB], F16, tag="rdh")
                        nc.scalar.copy(rdh[:], rdf[:])
                        pdb = psmp.tile([40, NB], F32, tag="sm")
                        nc.tensor.matmul(pdb[:], cst[:8, OCBC:OCBC + 40],
                                         rdh[:], start=True, stop=True)
                        cn = rtpool.tile([40, NB], F16, tag="cn")
                        nc.vector.tensor_mul(cn[:], e[:], pdb[:])

                        ps = ps80p.tile([80, NB], F32, tag="ps")
                        for c in range(C):
                            pcb = pbc.tile([128, NB], F32, tag="bc")
                            nc.tensor.matmul(
                                pcb[:], cst[:40, OBSEL + c * 128:OBSEL + (c + 1) * 128],
                                cn[:], start=True, stop=True)
                            tcm = rtpool.tile([128, NB], F16, tag=f"t{c}")
                            nc.vector.tensor_mul(tcm[:], u[:], pcb[:])
                            nc.tensor.matmul(
                                ps[:], cst[:, OWFL + c * 80:OWFL + (c + 1) * 80],
                                tcm[:], start=(c == 0), stop=(c == 4))

                    # ---- g = squash factor of s (per class) ----
                    ssq = rtpool.tile([80, NB], F16, tag="ssq")
                    nc.scalar.activation(ssq[:], ps[:], AF.Square)
                    pvq = psmp.tile([8, NB], F32, tag="sm")
                    nc.tensor.matmul(pvq[:], cst[:80, OJSEL:OJSEL + 8],
                                     ssq[:], start=True, stop=True)
                    lnv = smpool.tile([8, NB], F32, tag="lna")
                    nc.scalar.activation(lnv[:], pvq[:], AF.Ln)
                    l1v = smpool.tile([8, NB], F32, tag="lnb")
                    nc.scalar.activation(l1v[:], pvq[:], AF.Ln, bias=1.0)
                    zv = smpool.tile([8, NB], F32, tag="zz")
                    nc.vector.scalar_tensor_tensor(
                        zv[:], lnv[:], 0.5, l1v[:],
                        op0=AluOpType.mult, op1=AluOpType.subtract)
                    g = smpool.tile([8, NB], F16, tag="gg")
                    nc.scalar.activation(g[:], zv[:], AF.Exp)
                    pgb = psmp.tile([80, NB], F32, tag="sm")
                    nc.tensor.matmul(pgb[:], cst[:8, OGBC:OGBC + 80], g[:],
                                     start=True, stop=True)
                    gb = rtpool.tile([80, NB], F16, tag="gb")
                    nc.scalar.copy(gb[:], pgb[:])
                    v80 = rtpool.tile([80, NB], F16, tag="v80")
                    nc.vector.tensor_mul(v80[:], gb[:], ps[:])

                    if itr < 2:
                        # logits += sum_j uh*v
                        pat = psmp.tile([40, NB], F32, tag="sm")
                        for c in range(C):
                            pvb = pbc.tile([128, NB], F32, tag="bc")
                            nc.tensor.matmul(
                                pvb[:], cst[:80, OVBC + c * 128:OVBC + (c + 1) * 128],
                                v80[:], start=True, stop=True)
                            pr = rtpool.tile([128, NB], F16, tag=f"pr{c}")
                            nc.vector.tensor_mul(pr[:], uh[c][:], pvb[:])
                            nc.tensor.matmul(
                                pat[:], cst[:, OASEL + c * 40:OASEL + (c + 1) * 40],
                                pr[:], start=(c == 0), stop=(c == 4))
                        if itr == 0:
                            lg_sb = rtpool.tile([40, NB], F32, tag="lg")
                            nc.scalar.copy(lg_sb[:], pat[:])
                        else:
                            lg2 = rtpool.tile([40, NB], F32, tag="lg2")
                            nc.vector.tensor_add(lg2[:], lg_sb[:], pat[:])
                            lg_sb = lg2
                    else:
                        vo = vopool.tile([128, 4, 80], F32, tag="vo")
                        for q in range(4):
                            pvt = pbc.tile([128, 80], F16, tag="bc")
                            nc.tensor.transpose(
                                pvt[:], v80[:, q * 128:(q + 1) * 128],
                                cst[:80, OID80:OID80 + 80])
                            nc.scalar.copy(vo[:, q, :], pvt[:])
                        dst = v_d[it * NB:(it + 1) * NB, :].rearrange(
                            "(q p) j -> p q j", p=128)
                        nc.sync.dma_start(out=dst, in_=vo[:])

    nc.compile()
    return nc


_NC_CACHE: dict = {}


def _get_nc(nt: int) -> bass.Bass:
    if nt not in _NC_CACHE:
        _NC_CACHE[nt] = build_nc(nt)
    return _NC_CACHE[nt]


def make_in_maps(x, Wp, bp, W, nt: int = NT):
    """Shard + host-prep inputs for the SPMD launch (nt tiles per core)."""
    x = np.asarray(x, np.float32)
    cstb, cstf = build_consts(Wp, bp, W)
    bc = nt * NB
    maps = []
    for i in range(NCORES):
        xc = x[i * bc:(i + 1) * bc] if nt == NT else x[i * bc:(i + 1) * bc]
        maps.append({"xt": prep_x(xc), "cstb": cstb, "cstf": cstf})
    return maps


def kernel(x, Wp, bp, W):
    nc = _get_nc(NT)
    in_maps = make_in_maps(x, Wp, bp, W, NT)
    res = run_bass_kernel_spmd(nc, in_maps, list(range(NCORES)))
    out = np.concatenate([res.results[i]["vout"] for i in range(NCORES)], axis=0)
    return out.reshape(B, C, CD)


# revision 10
# speedup vs baseline: 1.9927x; 1.2479x over previous
"""Trainium2 Bass kernel for CapsuleLayer (nn_CapsuleLayer_45552423142009).

Computes, for x[B,768]:
  u = squash(x @ Wp + bp)            # [B, 8, 16]  (squash over last dim)
  u_hat[b,p,c,:] = u[b,p,:] @ W[p,c] # [B, 8, 5, 16]
  3 iterations of dynamic routing -> v [B, 5, 16]

Strategy: pure data-parallel over 8 NeuronCores (batch sharded 16384/core).
On-chip layout is "transposed": features on partitions, batch on the free
dim (512-wide tiles).  Key differences vs the v1 kernel:
  - x is transposed + cast to fp16 on the host, so the device does no
    PE transposes of x and reads half the HBM bytes.
  - all matmul operands are fp16 (1 cycle/row on the PE vs ~2 for f32r).
  - squash factors via exp/ln only (f = sqrt(q)/(1+q) = exp(.5*ln q -
    ln(1+q))), so the ACT engine stays on one table set (no ~2.7us
    ACT_TABLE_LOAD thrash), and softmax recip uses the fast DVE approx.
  - s/v are packed [80=(c,j), b] via M=80 accumulating selector matmuls:
    one vsq / g-broadcast / v-multiply per iteration instead of five.
  - routing logits accumulate directly in one pinned PSUM bank across
    iterations (PE start=False accumulation), read in place by Exp.
"""

import sys
import numpy as np

sys.path.insert(0, "/opt/trn_rl_repo")

from concourse import bass, bacc, mybir  # noqa: E402
from concourse import tile  # noqa: E402
from concourse.bass_utils import run_bass_kernel_spmd  # noqa: E402
from concourse.alu_op_type import AluOpType  # noqa: E402

F32 = mybir.dt.float32
F16 = mybir.dt.float16
AF = mybir.ActivationFunctionType

B = 131072
D = 768
P = 8
PD = 16
C = 5
CD = 16
NCORES = 8
BC = B // NCORES          # 16384 batch rows per core
NB = 512                  # batch columns per tile
NT = BC // NB             # 32 tiles

# fp16 constant blob column offsets
OWP = 0                   # [128, 768]   mm1 stationary (6 chunks of 128)
OWBD = 768                # [128, 640]   u_hat stationary, per class [128,128]
OS0 = 1408                # [128, 80]    itr0 s80 stationary (0.2*W, all classes)
OWFL = 1808               # [128, 400]   itr>0 s80 stationary (W per class)
OBSEL = 2208              # [40, 640]    cn -> (p,i) broadcast, per class
OASEL = 2848              # [128, 200]   agreement contraction, per class
OVBC = 3048               # [80, 640]    v80 -> (p,j) broadcast, per class
OSSB = 3688               # [128, 128]   fused: qb[(p,j)] = sum_j' usq[(p,j')]
OJGB = 3816               # [80, 80]     fused: qv[(c,j)] = sum_j' ssq[(c,j')]
OCSEL = 3896              # [40, 8]      sum over c at fixed p  (logits (c,p))
OCBC = 3904               # [8, 40]      broadcast p -> (c,p)
OID80 = 3944              # [80, 80]     identity (v transposes)
CW = 4024


def build_consts(Wp, bp, W):
    """Host-side packing of all selectors + weights into one fp16 blob
    plus the fp32 bias column."""
    Wp = np.asarray(Wp, np.float32)
    bp = np.asarray(bp, np.float32)
    W = np.asarray(W, np.float32)

    cst = np.zeros((128, CW), np.float32)

    # mm1 stationary: wp[(d_sub), k*128 + (p,o)] = Wp[d, p, o] with d = k*128+d_sub
    wp_flat = Wp.transpose(1, 0, 2).reshape(D, 128)            # [d, (p,o)]
    cst[:, OWP:OWP + 768] = wp_flat.reshape(6, 128, 128).transpose(1, 0, 2).reshape(128, 768)

    # u_hat stationary (block-diag over p): wbd[(p,i), c*128+(p,j)] = W[p,c,i,j]
    for p in range(P):
        for c in range(C):
            cst[p * 16:(p + 1) * 16, OWBD + c * 128 + p * 16:OWBD + c * 128 + (p + 1) * 16] = W[p, c]

    # itr0 s in one matmul: s0[(c,j)] = .2 sum_p u_p @ W_pc
    # os0[(p,i), c*16+j] = .2 * W[p,c,i,j]
    for c in range(C):
        for p in range(P):
            cst[p * 16:(p + 1) * 16, OS0 + c * 16:OS0 + (c + 1) * 16] += 0.2 * W[p, c]

    # itr>0 s: wfl[(p,i), c*80 + (c', j)] = W[p,c,i,j] * d_c'c
    for c in range(C):
        for p in range(P):
            cst[p * 16:(p + 1) * 16, OWFL + c * 80 + c * 16:OWFL + c * 80 + (c + 1) * 16] = W[p, c]

    # bsel[(c'p'), c*128 + (p,i)] = d_c'c d_p'p
    for c in range(C):
        for p in range(P):
            cst[c * 8 + p, OBSEL + c * 128 + p * 16:OBSEL + c * 128 + (p + 1) * 16] = 1.0

    # asel[(p,j), c*40 + (c',p')] = d_pp' d_c'c
    for c in range(C):
        for p in range(P):
            for j in range(CD):
                cst[p * 16 + j, OASEL + c * 40 + c * 8 + p] = 1.0

    # vbc[(c'',j'), c*128 + (p,j)] = d_c''c d_j'j
    for c in range(C):
        for p in range(P):
            for j in range(CD):
                cst[c * 16 + j, OVBC + c * 128 + p * 16 + j] = 1.0

    # fused sum-then-broadcast within 16-row groups: block all-ones
    for p in range(P):
        cst[p * 16:(p + 1) * 16, OSSB + p * 16:OSSB + (p + 1) * 16] = 1.0
    for c in range(C):
        cst[c * 16:(c + 1) * 16, OJGB + c * 16:OJGB + (c + 1) * 16] = 1.0

    # csel[(c,p), p'] / cbc[p', (c,p)]
    for c in range(C):
        for p in range(P):
            cst[c * 8 + p, OCSEL + p] = 1.0
            cst[p, OCBC + c * 8 + p] = 1.0

    cst[:80, OID80:OID80 + 80] = np.eye(80)

    bp_h = np.ascontiguousarray(bp.reshape(128, 1), dtype=np.float32)
    return np.ascontiguousarray(cst.astype(np.float16)), bp_h


def prep_x(x_core):
    """[bc, 768] fp32 -> tile-image [nt*128, 3072] fp16 with
    img[t*128+p, k*512+c] = x[t*512+c, k*128+p] (pre-transposed)."""
    bc = x_core.shape[0]
    nt = bc // NB
    xi = x_core.reshape(nt, NB, 6, 128).transpose(0, 3, 2, 1)  # [t, p, k, c]
    return np.ascontiguousarray(xi.reshape(nt * 128, 6 * NB), dtype=np.float16)


def build_nc(nt: int = NT) -> bass.Bass:
    bc = nt * NB
    nc = bacc.Bacc(None)

    x_d = nc.declare_dram_parameter("xt", [nt * 128, 6 * NB], F16, isOutput=False)
    cb_d = nc.declare_dram_parameter("cstb", [128, CW], F16, isOutput=False)
    cf_d = nc.declare_dram_parameter("cstf", [128, 1], F32, isOutput=False)
    v_d = nc.declare_dram_parameter("vout", [bc, C * CD], F32, isOutput=True)

    with tile.TileContext(nc) as tc, nc.allow_low_precision(reason="fp16 compute"):
        with (
            tc.sbuf_pool(name="const", bufs=1) as cpool,
            tc.sbuf_pool(name="xin", bufs=3) as xpool,
            tc.sbuf_pool(name="mid", bufs=2) as mpool,
            tc.sbuf_pool(name="uh", bufs=2) as uhpool,
            tc.sbuf_pool(name="rt", bufs=2) as rtpool,
            tc.sbuf_pool(name="sm", bufs=3) as smpool,
            tc.sbuf_pool(name="vo", bufs=2) as vopool,
            tc.psum_pool(name="pfront", bufs=2) as pfr,
            tc.psum_pool(name="pbcast", bufs=2) as pbc,
            tc.psum_pool(name="ps80", bufs=2) as ps80p,
            tc.psum_pool(name="psm", bufs=2) as psmp,
        ):
            # ---- pin the one ACT table set holding Ln+Exp+Square+Copy+
            # Identity, so the first-fit table chooser never thrashes ----
            nc.scalar.add_instruction(mybir.InstLoadActFuncSet(
                name=nc.get_next_instruction_name(), ins=[], outs=[],
                act_func_set_id=6))  # natural_log_exp_and_others

            # ---- constants: one DMA each, staged through DVE ----
            cst0 = cpool.tile([128, CW], F16)
            nc.sync.dma_start(out=cst0[:], in_=cb_d[:])
            cst = cpool.tile([128, CW], F16)
            nc.vector.tensor_copy(cst[:], cst0[:])
            bp0 = cpool.tile([128, 1], F32)
            nc.sync.dma_start(out=bp0[:], in_=cf_d[:])
            bp_sb = cpool.tile([128, 1], F32)
            nc.vector.tensor_copy(bp_sb[:], bp0[:])

            for it in range(nt):
                xts = xpool.tile([128, 6 * NB], F16, tag="xin")
                nc.sync.dma_start(out=xts[:], in_=x_d[it * 128:(it + 1) * 128, :])

                # ---- mm1: u_pre[(p,o), b] = Wp^T x^T + bp ----
                pm = pfr.tile([128, NB], F32, tag="fr")
                for k in range(6):
                    nc.tensor.matmul(
                        pm[:], cst[:, OWP + k * 128:OWP + (k + 1) * 128],
                        xts[:, k * NB:(k + 1) * NB],
                        start=(k == 0), stop=(k == 5))
                u_pre = mpool.tile([128, NB], F16, tag="upre")
                nc.scalar.activation(u_pre[:], pm[:], AF.Identity,
                                     bias=bp_sb[:], scale=1.0)
                usq = mpool.tile([128, NB], F16, tag="usq")
                nc.scalar.activation(usq[:], pm[:], AF.Square,
                                     bias=bp_sb[:], scale=1.0)

                # ---- squash factor f = exp(.5 ln q - ln(1+q)), computed
                # directly at (p,j) width via the fused sum+bcast selector ----
                pq = pfr.tile([128, NB], F32, tag="fr")
                nc.tensor.matmul(pq[:], cst[:, OSSB:OSSB + 128], usq[:],
                                 start=True, stop=True)
                lnq = smpool.tile([128, NB], F32, tag="lna")
                nc.scalar.activation(lnq[:], pq[:], AF.Ln)
                l1q = smpool.tile([128, NB], F32, tag="lnb")
                nc.scalar.activation(l1q[:], pq[:], AF.Ln, bias=1.0)
                fz = smpool.tile([128, NB], F16, tag="ff")
                nc.vector.scalar_tensor_tensor(
                    fz[:], lnq[:], 0.5, l1q[:],
                    op0=AluOpType.mult, op1=AluOpType.subtract)
                fb = smpool.tile([128, NB], F16, tag="fb")
                nc.scalar.activation(fb[:], fz[:], AF.Exp)
                u = mpool.tile([128, NB], F16, tag="uu")
                nc.vector.tensor_mul(u[:], u_pre[:], fb[:])

                # ---- u_hat per class ----
                uh = []
                for c in range(C):
                    puh = pfr.tile([128, NB], F32, tag="fr")
                    nc.tensor.matmul(
                        puh[:], cst[:, OWBD + c * 128:OWBD + (c + 1) * 128],
                        u[:], start=True, stop=True)
                    uhc = uhpool.tile([128, NB], F16, tag=f"uh{c}")
                    if c in (0, 1, 3):
                        nc.scalar.copy(uhc[:], puh[:])  # ACT/DVE balance
                    else:
                        nc.vector.tensor_copy(uhc[:], puh[:])
                    uh.append(uhc)

                lg_sb = None

                for itr in range(3):
                    if itr == 0:
                        ps = ps80p.tile([80, NB], F32, tag="ps")
                        nc.tensor.matmul(ps[:], cst[:, OS0:OS0 + 80], u[:],
                                         start=True, stop=True)
                    else:
                        # softmax over classes of logits [ (c,p), b ]
                        e = rtpool.tile([40, NB], F16, tag="ee")
                        nc.scalar.activation(e[:], lg_sb[:], AF.Exp)
                        pden = psmp.tile([8, NB], F32, tag="sm")
                        nc.tensor.matmul(pden[:], cst[:40, OCSEL:OCSEL + 8],
                                         e[:], start=True, stop=True)
                        rdf = smpool.tile([8, NB], F32, tag="rdf")
                        nc.vector.reciprocal_approx_fast(out=rdf[:], in_=pden[:])
                        rdh = smpool.tile([8, NB], F16, tag="rdh")
                        nc.vector.tensor_copy(rdh[:], rdf[:])
                        pdb = psmp.tile([40, NB], F32, tag="sm")
                        nc.tensor.matmul(pdb[:], cst[:8, OCBC:OCBC + 40],
                                         rdh[:], start=True, stop=True)
                        cn = rtpool.tile([40, NB], F16, tag="cn")
                        nc.vector.tensor_mul(cn[:], e[:], pdb[:])

                        ps = ps80p.tile([80, NB], F32, tag="ps")
                        for c in range(C):
                            pcb = pbc.tile([128, NB], F32, tag="bc")
                            nc.tensor.matmul(
                                pcb[:], cst[:40, OBSEL + c * 128:OBSEL + (c + 1) * 128],
                                cn[:], start=True, stop=True)
                            tcm = rtpool.tile([128, NB], F16, tag=f"t{c}")
                            nc.vector.tensor_mul(tcm[:], u[:], pcb[:])
                            nc.tensor.matmul(
                                ps[:], cst[:, OWFL + c * 80:OWFL + (c + 1) * 80],
                                tcm[:], start=(c == 0), stop=(c == 4))

                    # ---- g = squash factor of s, computed directly at
                    # (c,j) width via the fused sum+bcast selector ----
                    ssq = rtpool.tile([80, NB], F16, tag="ssq")
                    nc.scalar.activation(ssq[:], ps[:], AF.Square)
                    pvq = psmp.tile([80, NB], F32, tag="sm")
                    nc.tensor.matmul(pvq[:], cst[:80, OJGB:OJGB + 80],
                                     ssq[:], start=True, stop=True)
                    lnv = smpool.tile([80, NB], F32, tag="lnc")
                    nc.scalar.activation(lnv[:], pvq[:], AF.Ln)
                    l1v = smpool.tile([80, NB], F32, tag="lnd")
                    nc.scalar.activation(l1v[:], pvq[:], AF.Ln, bias=1.0)
                    zv = smpool.tile([80, NB], F16, tag="zv")
                    nc.vector.scalar_tensor_tensor(
                        zv[:], lnv[:], 0.5, l1v[:],
                        op0=AluOpType.mult, op1=AluOpType.subtract)
                    gb = rtpool.tile([80, NB], F16, tag="gb")
                    nc.scalar.activation(gb[:], zv[:], AF.Exp)
                    v80 = rtpool.tile([80, NB], F16, tag="v80")
                    nc.vector.tensor_mul(v80[:], gb[:], ps[:])

                    if itr < 2:
                        # logits += sum_j uh*v
                        pat = psmp.tile([40, NB], F32, tag="sm")
                        for c in range(C):
                            pvb = pbc.tile([128, NB], F32, tag="bc")
                            nc.tensor.matmul(
                                pvb[:], cst[:80, OVBC + c * 128:OVBC + (c + 1) * 128],
                                v80[:], start=True, stop=True)
                            pr = rtpool.tile([128, NB], F16, tag=f"pr{c}")
                            nc.vector.tensor_mul(pr[:], uh[c][:], pvb[:])
                            nc.tensor.matmul(
                                pat[:], cst[:, OASEL + c * 40:OASEL + (c + 1) * 40],
                                pr[:], start=(c == 0), stop=(c == 4))
                        if itr == 0:
                            lg_sb = rtpool.tile([40, NB], F32, tag="lg")
                            nc.scalar.copy(lg_sb[:], pat[:])
                        else:
                            lg2 = rtpool.tile([40, NB], F32, tag="lg2")
                            nc.vector.tensor_add(lg2[:], lg_sb[:], pat[:])
                            lg_sb = lg2
                    else:
                        vo = vopool.tile([128, 4, 80], F32, tag="vo")
                        for q in range(4):
                            pvt = pbc.tile([128, 80], F16, tag="bc")
                            nc.tensor.transpose(
                                pvt[:], v80[:, q * 128:(q + 1) * 128],
                                cst[:80, OID80:OID80 + 80])
                            nc.scalar.copy(vo[:, q, :], pvt[:])
                        dst = v_d[it * NB:(it + 1) * NB, :].rearrange(
                            "(q p) j -> p q j", p=128)
                        nc.sync.dma_start(out=dst, in_=vo[:])

    nc.compile()
    return nc


_NC_CACHE: dict = {}


def _get_nc(nt: int) -> bass.Bass:
    if nt not in _NC_CACHE:
        _NC_CACHE[nt] = build_nc(nt)
    return _NC_CACHE[nt]


def make_in_maps(x, Wp, bp, W, nt: int = NT):
    """Shard + host-prep inputs for the SPMD launch (nt tiles per core)."""
    x = np.asarray(x, np.float32)
    cstb, cstf = build_consts(Wp, bp, W)
    bc = nt * NB
    maps = []
    for i in range(NCORES):
        xc = x[i * bc:(i + 1) * bc] if nt == NT else x[i * bc:(i + 1) * bc]
        maps.append({"xt": prep_x(xc), "cstb": cstb, "cstf": cstf})
    return maps


def kernel(x, Wp, bp, W):
    nc = _get_nc(NT)
    in_maps = make_in_maps(x, Wp, bp, W, NT)
    res = run_bass_kernel_spmd(nc, in_maps, list(range(NCORES)))
    out = np.concatenate([res.results[i]["vout"] for i in range(NCORES)], axis=0)
    return out.reshape(B, C, CD)
